# revision 2
# baseline (speedup 1.0000x reference)
"""3-layer GCN (message passing) on 8 TRN2 NeuronCores.

Strategy: shard destination nodes across cores (graph parallel). All edge
normalization (deg, dinv, per-edge norm = dinv_s*w*dinv_d) is precomputed on
the host. Per layer, per core:
  HP = (prev @ W)^T computed locally on the node shard (PE), rows
  transposed (PE) + stored; two AllGathers (A/B half tables, bf16) publish
  all source rows. Per group of 4 dst blocks: two SWDGE dma_gather calls
  (lo/hi halves so gather indices fit int16) pull source rows; the weighted
  one-hot S_w[e,d] = (dstloc[e]==iota[d]) * norm[e] is built on DVE with two
  broadcast-AP ops (no DRAM traffic); PE accumulates out^T = M^T @ S_w per
  block into a group PSUM bank. Epilogue adds the self-loop term
  dinv^2 * HP and bias, applies leaky-relu, and the next layer's dense
  matmul + transpose + hcur stores are fused into the same group loop so the
  next AllGather's inputs are ready the moment the layer ends.
"""

import numpy as np

import concourse.bacc as bacc
import concourse.mybir as mybir
from concourse.tile import TileContext
from concourse.bass_utils import run_bass_kernel_spmd

try:
    import ml_dtypes

    BF16 = ml_dtypes.bfloat16
except ImportError:  # pragma: no cover
    BF16 = None

N_CORES = 8
D = 128
NEG_SLOPE = 0.1
G_BLOCKS = 4  # dst blocks per matmul group / gather call pair
LEAKY_VIA_PRELU = True  # sim validation sets False (Prelu not in CoreSim)


def _ceil_div(a, b):
    return (a + b - 1) // b


def _wrap_idx(idx):
    """[cnt] int16 -> [128, cnt//16] wrapped layout (16-partition, replicated x8)."""
    cnt = idx.shape[0]
    assert cnt % 16 == 0
    w = idx.reshape(cnt // 16, 16).T  # [16, cnt//16]
    return np.tile(w, (8, 1)).astype(np.int16)  # [128, cnt//16]


def _preprocess(x, edge_index, edge_attr, edge_type, edge_type_scale):
    """Host-side normalization + sharding/layout. Returns (meta, per-core inputs)."""
    N = x.shape[0]
    assert N % N_CORES == 0
    per = N // N_CORES
    nb = _ceil_div(per, 128)
    per_pad = nb * 128
    SA = max(16, ((per // 2) // 16) * 16)
    SB = per - SA
    assert SA * N_CORES <= 32768 and SB * N_CORES <= 32768

    src_f = np.asarray(edge_index[0], dtype=np.int64)
    dst_f = np.asarray(edge_index[1], dtype=np.int64)
    ets = np.asarray(edge_type_scale, dtype=np.float64)
    w = ets[np.asarray(edge_type, dtype=np.int64)] * np.asarray(
        edge_attr, dtype=np.float64
    )
    deg = np.bincount(dst_f, weights=w, minlength=N) + 1.0  # +1 = self loop
    dinv = 1.0 / np.sqrt(deg)
    norm = (dinv[src_f] * w * dinv[dst_f]).astype(np.float32)
    dinv2 = (dinv * dinv).astype(np.float32)

    core = dst_f // per
    ldst = dst_f - core * per
    blk = ldst >> 7
    slot = ldst & 127
    src_c = src_f // per
    src_r = src_f - src_c * per
    half = (src_r >= SA).astype(np.int64)
    gidx = np.where(half == 0, src_c * SA + src_r, src_c * SB + (src_r - SA))

    counts = np.zeros((N_CORES, nb, 2), dtype=np.int64)
    per_core_e = []
    for c in range(N_CORES):
        m = core == c
        order = np.lexsort((src_f[m], half[m], blk[m]))
        per_core_e.append(
            dict(src=gidx[m][order], slot=slot[m][order], w=norm[m][order])
        )
        counts[c] = np.bincount(
            blk[m] * 2 + half[m], minlength=nb * 2
        ).reshape(nb, 2)

    # common padded schedule: tiles per (block, half), maxed over cores
    tiles_bh = np.maximum(1, _ceil_div(counts.max(axis=0), 128))  # [nb, 2]
    pad_bh = tiles_bh * 128

    groups = [list(range(g, min(g + G_BLOCKS, nb))) for g in range(0, nb, G_BLOCKS)]
    slot_off = np.zeros((nb, 2), dtype=np.int64)
    call_cnt = []  # per (group, half): total padded count = gather call size
    off = 0
    for g in groups:
        for h in (0, 1):
            c0 = off
            for b in g:
                slot_off[b, h] = off
                off += pad_bh[b, h]
            call_cnt.append(off - c0)
    totslot = off
    T = totslot // 128

    ins = []
    for c in range(N_CORES):
        pc = per_core_e[c]
        idx_sl = np.zeros(totslot, dtype=np.int16)
        dst_sl = np.zeros(totslot, dtype=np.float32)
        w_sl = np.zeros(totslot, dtype=np.float32)
        starts = np.zeros((nb, 2), dtype=np.int64)
        starts.reshape(-1)[1:] = np.cumsum(counts[c].reshape(-1))[:-1]
        for b in range(nb):
            for h in (0, 1):
                n = counts[c, b, h]
                if n:
                    s0 = starts[b, h]
                    o = slot_off[b, h]
                    idx_sl[o : o + n] = pc["src"][s0 : s0 + n].astype(np.int16)
                    dst_sl[o : o + n] = pc["slot"][s0 : s0 + n]
                    w_sl[o : o + n] = pc["w"][s0 : s0 + n]

        wrapped = []
        off2 = 0
        for cc in call_cnt:
            wrapped.append(_wrap_idx(idx_sl[off2 : off2 + cc]))
            off2 += cc
        idx_w = np.concatenate(wrapped, axis=1)  # [128, totslot//16]

        col = lambda a: np.ascontiguousarray(a.reshape(T, 128).T)  # [128, T]
        xt = np.zeros((128, per_pad), dtype=np.float32)
        xt[:, :per] = np.asarray(x[c * per : (c + 1) * per], dtype=np.float32).T
        d2 = np.zeros((per_pad,), dtype=np.float32)
        d2[:per] = dinv2[c * per : (c + 1) * per]
        ins.append(
            dict(
                IDX=idx_w,
                DSTLOC=col(dst_sl).astype(BF16),
                WCOL=col(w_sl).astype(BF16),
                XT=xt,
                DINV2B=np.ascontiguousarray(
                    np.broadcast_to(d2[None, :], (128, per_pad))
                ).astype(BF16),
            )
        )

    meta = dict(
        N=N, per=per, nb=nb, per_pad=per_pad, SA=SA, T=T, totslot=totslot,
        groups=groups, call_cnt=call_cnt, tiles_bh=tiles_bh, slot_off=slot_off,
    )
    return meta, ins


def _build(meta):
    per = meta["per"]
    nb = meta["nb"]
    per_pad = meta["per_pad"]
    SA = meta["SA"]
    SB = per - SA
    T = meta["T"]
    totslot = meta["totslot"]
    groups = meta["groups"]
    call_cnt = meta["call_cnt"]
    tiles_bh = meta["tiles_bh"]
    slot_off = meta["slot_off"]

    f32 = mybir.dt.float32
    bf16 = mybir.dt.bfloat16
    i16 = mybir.dt.int16

    call_base = [sum(call_cnt[:i]) for i in range(len(call_cnt))]
    maxw16 = max(c // 16 for c in call_cnt)
    maxw128 = max(c // 128 for c in call_cnt)
    ag_gi = ((SA - 1) >> 7) // G_BLOCKS  # group whose stores complete hcurA

    nc = bacc.Bacc("TRN2", num_devices=N_CORES, num_swdge_queues=4,
                   dynamic_dma_scratch_size=32768)

    t_idx = nc.dram_tensor("IDX", [128, totslot // 16], i16, kind="ExternalInput")
    t_dstloc = nc.dram_tensor("DSTLOC", [128, T], bf16, kind="ExternalInput")
    t_wcol = nc.dram_tensor("WCOL", [128, T], bf16, kind="ExternalInput")
    t_xt = nc.dram_tensor("XT", [128, per_pad], f32, kind="ExternalInput")
    t_dinv2 = nc.dram_tensor("DINV2B", [128, per_pad], bf16, kind="ExternalInput")
    t_W = [
        nc.dram_tensor(f"W{i}", [128, 128], f32, kind="ExternalInput") for i in (1, 2, 3)
    ]
    t_b = [
        nc.dram_tensor(f"b{i}", [128, 1], f32, kind="ExternalInput") for i in (1, 2, 3)
    ]
    t_iota_b = nc.dram_tensor("IOTAB", [128, 128], bf16, kind="ExternalInput")
    t_ident = nc.dram_tensor("IDENT", [128, 128], f32, kind="ExternalInput")
    t_identb = nc.dram_tensor("IDENTB", [128, 128], bf16, kind="ExternalInput")
    t_out = nc.dram_tensor("OUT", [per, 128], f32, kind="ExternalOutput")

    hcurA = [
        nc.dram_tensor(f"hcurA{l}", [SA, 128], bf16, kind="Internal") for l in range(3)
    ]
    hcurB = [
        nc.dram_tensor(f"hcurB{l}", [SB, 128], bf16, kind="Internal") for l in range(3)
    ]
    hfullA = [
        nc.dram_tensor(
            f"hfullA{l}", [N_CORES * SA, 128], bf16, kind="Internal",
            addr_space="Shared",
        )
        for l in range(3)
    ]
    hfullB = [
        nc.dram_tensor(
            f"hfullB{l}", [N_CORES * SB, 128], bf16, kind="Internal",
            addr_space="Shared",
        )
        for l in range(3)
    ]
    rg = [list(range(N_CORES))]

    with TileContext(nc) as tc:
        with (
            tc.tile_pool(name="persist", bufs=1) as pp,
            tc.tile_pool(name="work", bufs=2) as wp,
            tc.tile_pool(name="mp", bufs=2) as mp,
            tc.tile_pool(name="sp", bufs=2) as sp,
            tc.tile_pool(name="psg", bufs=2, space="PSUM") as psg,
            tc.tile_pool(name="psd", bufs=2, space="PSUM") as psd,
            tc.tile_pool(name="pst", bufs=2, space="PSUM") as pst,
        ):
            # ---------- persistent loads ----------
            DSTLOC = pp.tile([128, T], bf16, tag="DSTLOC")
            nc.sync.dma_start(DSTLOC[:, :], t_dstloc[:, :])
            WCOL = pp.tile([128, T], bf16, tag="WCOL")
            nc.sync.dma_start(WCOL[:, :], t_wcol[:, :])
            IOTAB = pp.tile([128, 128], bf16, tag="IOTAB")
            nc.sync.dma_start(IOTAB[:, :], t_iota_b[:, :])
            IDENT = pp.tile([128, 128], f32, tag="IDENT")
            nc.sync.dma_start(IDENT[:, :], t_ident[:, :])
            IDENTB = pp.tile([128, 128], bf16, tag="IDENTB")
            nc.sync.dma_start(IDENTB[:, :], t_identb[:, :])
            DINV2B = pp.tile([128, per_pad], bf16, tag="DINV2B")
            nc.sync.dma_start(DINV2B[:, :], t_dinv2[:, :])
            W = []
            B = []
            for i in range(3):
                Wt = pp.tile([128, 128], f32, tag=f"W{i}")
                nc.sync.dma_start(Wt[:, :], t_W[i][:, :])
                W.append(Wt)
                Bt = pp.tile([128, 1], f32, tag=f"B{i}")
                nc.sync.dma_start(Bt[:, :], t_b[i][:, :])
                B.append(Bt)

            HP = pp.tile([128, per_pad], bf16, tag="HP")
            HOUT = pp.tile([128, per_pad], f32, tag="HOUT")

            def store_rows(rt, cb, l):
                r0 = cb * 128
                r1 = min(per, r0 + 128)
                if r1 <= SA:
                    nc.sync.dma_start(hcurA[l][r0:r1, :], rt[0 : r1 - r0, :])
                elif r0 >= SA:
                    nc.sync.dma_start(
                        hcurB[l][r0 - SA : r1 - SA, :], rt[0 : r1 - r0, :]
                    )
                else:
                    nc.sync.dma_start(hcurA[l][r0:SA, :], rt[0 : SA - r0, :])
                    nc.sync.dma_start(
                        hcurB[l][0 : r1 - SA, :], rt[SA - r0 : r1 - r0, :]
                    )

            def all_gather(h, l):
                cur, full = (hcurA, hfullA) if h == 0 else (hcurB, hfullB)
                nc.gpsimd.collective_compute(
                    "AllGather", mybir.AluOpType.bypass,
                    ins=[cur[l][:, :]], outs=[full[l][:, :]],
                    replica_groups=rg,
                )

            def producer_group(g, l, src):
                """Transpose src chunk of HP/HOUT rows into hcur[l] stores."""
                for cb in g:
                    if src is HP:
                        pt = pst.tile([128, 128], bf16, tag="pt")
                        nc.tensor.transpose(
                            pt[:, :], src[:, cb * 128 : (cb + 1) * 128], IDENTB[:, :]
                        )
                        rt = wp.tile([128, 128], bf16, tag="rowb")
                        nc.vector.tensor_copy(rt[:, :], pt[:, :])
                        store_rows(rt, cb, l)
                    else:
                        ptf = pst.tile([128, 128], f32, tag="ptf", bufs=1)
                        nc.tensor.transpose(
                            ptf[:, :], src[:, cb * 128 : (cb + 1) * 128], IDENT[:, :]
                        )
                        rf = wp.tile([128, 128], f32, tag="rowf")
                        nc.vector.tensor_copy(rf[:, :], ptf[:, :])
                        r0 = cb * 128
                        r1 = min(per, r0 + 128)
                        nc.sync.dma_start(t_out[r0:r1, :], rf[0 : r1 - r0, :])

            # ---------- layer-0 producer: HP = (x @ W1)^T, publish rows ----
            for gi, g in enumerate(groups):
                g0 = g[0] * 128
                gw = len(g) * 128
                xc = wp.tile([128, 512], f32, tag="xc")
                nc.sync.dma_start(xc[:, :gw], t_xt[:, g0 : g0 + gw])
                ph = psd.tile([128, 512], f32, tag="pd")
                nc.tensor.matmul(ph[:, :gw], W[0][:, :], xc[:, :gw], start=True, stop=True)
                nc.vector.tensor_copy(HP[:, g0 : g0 + gw], ph[:, :gw])
                producer_group(g, 0, HP)
                if gi == ag_gi:
                    all_gather(0, 0)
            all_gather(1, 0)

            # ---------- layers ----------
            for l in range(3):
                for gi, g in enumerate(groups):
                    g0 = g[0] * 128
                    gw = len(g) * 128
                    mts = {}
                    for h in (0, 1):
                        ci = 2 * gi + h
                        cnt = call_cnt[ci]
                        ntl = cnt // 128
                        woff = call_base[ci] // 16
                        idxt = wp.tile([128, maxw16], i16, tag="idx", bufs=4)
                        nc.sync.dma_start(
                            idxt[:, : cnt // 16], t_idx[:, woff : woff + cnt // 16]
                        )
                        mt = mp.tile([128, maxw128, 128], bf16, tag=f"m{h}")
                        src_tab = hfullA[l][:, :] if h == 0 else hfullB[l][:, :]
                        nc.gpsimd.dma_gather(
                            mt[:, :ntl, :], src_tab, idxt[:, : cnt // 16],
                            num_idxs=cnt, num_idxs_reg=cnt, elem_size=128,
                            single_packet=False, queue_num=ci % 4,
                        )
                        mts[h] = mt
                    sws = {}
                    for h in (0, 1):
                        ci = 2 * gi + h
                        cnt = call_cnt[ci]
                        ntl = cnt // 128
                        b128 = call_base[ci] // 128
                        sw = sp.tile([128, maxw128, 128], bf16, tag=f"s{h}")
                        d_b = (
                            DSTLOC[:, b128 : b128 + ntl]
                            .unsqueeze(2)
                            .to_broadcast([128, ntl, 128])
                        )
                        i_b = IOTAB[:, :].unsqueeze(1).to_broadcast([128, ntl, 128])
                        w_b = (
                            WCOL[:, b128 : b128 + ntl]
                            .unsqueeze(2)
                            .to_broadcast([128, ntl, 128])
                        )
                        nc.vector.tensor_tensor(
                            sw[:, :ntl, :], d_b, i_b, op=mybir.AluOpType.is_equal
                        )
                        nc.vector.tensor_tensor(
                            sw[:, :ntl, :], sw[:, :ntl, :], w_b,
                            op=mybir.AluOpType.mult,
                        )
                        sws[h] = sw

                    pg = psg.tile([128, 512], f32, tag="pg")
                    for bi, b in enumerate(g):
                        seq = []
                        for h in (0, 1):
                            j0 = (slot_off[b][h] - call_base[2 * gi + h]) // 128
                            seq += [(h, j) for j in range(j0, j0 + int(tiles_bh[b][h]))]
                        for k, (h, j) in enumerate(seq):
                            nc.tensor.matmul(
                                pg[:, bi * 128 : (bi + 1) * 128],
                                mts[h][:, j, :], sws[h][:, j, :],
                                start=(k == 0), stop=(k == len(seq) - 1),
                            )

                    # epilogue: self-loop + bias + leaky
                    tmp = wp.tile([128, 512], f32, tag="tmp")
                    nc.vector.tensor_tensor(
                        tmp[:, :gw], HP[:, g0 : g0 + gw], DINV2B[:, g0 : g0 + gw],
                        op=mybir.AluOpType.mult,
                    )
                    ep = wp.tile([128, 512], f32, tag="ep")
                    nc.vector.tensor_tensor(
                        ep[:, :gw], pg[:, :gw], tmp[:, :gw], op=mybir.AluOpType.add
                    )
                    if l < 2:
                        if LEAKY_VIA_PRELU:
                            nc.scalar.activation(
                                HOUT[:, g0 : g0 + gw], ep[:, :gw],
                                mybir.ActivationFunctionType.Prelu,
                                bias=B[l][:, 0:1], scale=1.0, alpha=NEG_SLOPE,
                            )
                        else:
                            t2 = wp.tile([128, 512], f32, tag="ep2")
                            nc.scalar.activation(
                                t2[:, :gw], ep[:, :gw],
                                mybir.ActivationFunctionType.Identity,
                                bias=B[l][:, 0:1], scale=1.0,
                            )
                            t3 = wp.tile([128, 512], f32, tag="ep3")
                            nc.vector.tensor_scalar_mul(
                                t3[:, :gw], t2[:, :gw], NEG_SLOPE
                            )
                            nc.vector.tensor_tensor(
                                HOUT[:, g0 : g0 + gw], t2[:, :gw], t3[:, :gw],
                                op=mybir.AluOpType.max,
                            )
                        # fused next-layer producer for this chunk
                        ph = psd.tile([128, 512], f32, tag="pd")
                        nc.tensor.matmul(
                            ph[:, :gw], W[l + 1][:, :], HOUT[:, g0 : g0 + gw],
                            start=True, stop=True,
                        )
                        nc.vector.tensor_copy(HP[:, g0 : g0 + gw], ph[:, :gw])
                        producer_group(g, l + 1, HP)
                    else:
                        nc.scalar.activation(
                            HOUT[:, g0 : g0 + gw], ep[:, :gw],
                            mybir.ActivationFunctionType.Identity,
                            bias=B[l][:, 0:1], scale=1.0,
                        )
                        producer_group(g, l, HOUT)

                if l < 2:
                    all_gather(0, l + 1)
                    all_gather(1, l + 1)

    nc.compile()
    return nc


_CACHE = {}


def kernel(
    x,
    edge_index,
    edge_attr,
    edge_type,
    edge_type_scale,
    W1,
    b1,
    W2,
    b2,
    W3,
    b3,
):
    x = np.asarray(x)
    N = x.shape[0]
    meta, per_core = _preprocess(
        np.asarray(x), np.asarray(edge_index), np.asarray(edge_attr),
        np.asarray(edge_type), np.asarray(edge_type_scale),
    )

    key = (N, meta["T"], tuple(meta["call_cnt"]))
    if key not in _CACHE:
        _CACHE[key] = _build(meta)
    nc = _CACHE[key]

    iota_f = np.tile(np.arange(128, dtype=np.float32)[None, :], (128, 1))
    ident = np.eye(128, dtype=np.float32)
    common = dict(
        W1=np.asarray(W1, np.float32),
        W2=np.asarray(W2, np.float32),
        W3=np.asarray(W3, np.float32),
        b1=np.asarray(b1, np.float32).reshape(D, 1),
        b2=np.asarray(b2, np.float32).reshape(D, 1),
        b3=np.asarray(b3, np.float32).reshape(D, 1),
        IOTAB=iota_f.astype(BF16),
        IDENT=ident,
        IDENTB=ident.astype(BF16),
    )
    in_maps = []
    for c in range(N_CORES):
        m = dict(common)
        m.update(per_core[c])
        in_maps.append(m)

    res = run_bass_kernel_spmd(
        nc, in_maps, core_ids=list(range(N_CORES)), **_RUN_KWARGS
    )
    _LAST_RESULT.clear()
    _LAST_RESULT["exec_time_ns"] = res.exec_time_ns
    _LAST_RESULT["profile_json"] = res.profile_json
    out = np.concatenate([res.results[c]["OUT"] for c in range(N_CORES)], axis=0)
    return out.astype(np.float32)


_RUN_KWARGS = {}  # test harness can set {"trace": True, "tmpdir": ...}
_LAST_RESULT = {}


# revision 13
# speedup vs baseline: 1.2464x; 1.2464x over previous
"""3-layer GCN (message passing) on 8 TRN2 NeuronCores.

Strategy: shard destination nodes across cores (graph parallel). All edge
normalization (deg, dinv, per-edge norm = dinv_s*w*dinv_d) is precomputed on
the host. Per layer, per core:
  HP = (prev @ W)^T computed locally on the node shard (PE), rows
  transposed (PE) + stored; two AllGathers (A/B half tables, bf16) publish
  all source rows. Per group of 4 dst blocks: two SWDGE dma_gather calls
  (lo/hi halves so gather indices fit int16) pull source rows; the weighted
  one-hot S_w[e,d] = (dstloc[e]==iota[d]) * norm[e] is built on DVE with two
  broadcast-AP ops (no DRAM traffic); PE accumulates out^T = M^T @ S_w per
  block into a group PSUM bank. Epilogue adds the self-loop term
  dinv^2 * HP and bias, applies leaky-relu, and the next layer's dense
  matmul + transpose + hcur stores are fused into the same group loop so the
  next AllGather's inputs are ready the moment the layer ends.
"""

import numpy as np

import concourse.bacc as bacc
import concourse.mybir as mybir
from concourse.tile import TileContext
from concourse.bass_utils import run_bass_kernel_spmd

try:
    import ml_dtypes

    BF16 = ml_dtypes.bfloat16
except ImportError:  # pragma: no cover
    BF16 = None

N_CORES = 8
D = 128
NEG_SLOPE = 0.1
G_BLOCKS = 4  # dst blocks per matmul group / gather call pair
LEAKY_VIA_PRELU = True  # sim validation sets False (Prelu not in CoreSim)
SINGLE_PACKET = False


def _ceil_div(a, b):
    return (a + b - 1) // b


def _wrap_idx(idx):
    """[cnt] int16 -> [128, cnt//16] wrapped layout (16-partition, replicated x8)."""
    cnt = idx.shape[0]
    assert cnt % 16 == 0
    w = idx.reshape(cnt // 16, 16).T  # [16, cnt//16]
    return np.tile(w, (8, 1)).astype(np.int16)  # [128, cnt//16]


def _preprocess(x, edge_index, edge_attr, edge_type, edge_type_scale):
    """Host-side normalization + sharding/layout. Returns (meta, per-core inputs)."""
    N = x.shape[0]
    assert N % N_CORES == 0
    per = N // N_CORES
    nb = _ceil_div(per, 128)
    per_pad = nb * 128
    SA = max(16, ((per // 2) // 16) * 16)
    SB = per - SA
    assert SA * N_CORES <= 32768 and SB * N_CORES <= 32768

    src_f = np.asarray(edge_index[0], dtype=np.int64)
    dst_f = np.asarray(edge_index[1], dtype=np.int64)
    ets = np.asarray(edge_type_scale, dtype=np.float64)
    w = ets[np.asarray(edge_type, dtype=np.int64)] * np.asarray(
        edge_attr, dtype=np.float64
    )
    deg = np.bincount(dst_f, weights=w, minlength=N) + 1.0  # +1 = self loop
    dinv = 1.0 / np.sqrt(deg)
    norm = (dinv[src_f] * w * dinv[dst_f]).astype(np.float32)
    dinv2 = (dinv * dinv).astype(np.float32)

    core = dst_f // per
    ldst = dst_f - core * per
    blk = ldst >> 7
    slot = ldst & 127
    src_c = src_f // per
    src_r = src_f - src_c * per
    half = (src_r >= SA).astype(np.int64)
    gidx = np.where(half == 0, src_c * SA + src_r, src_c * SB + (src_r - SA))

    counts = np.zeros((N_CORES, nb, 2), dtype=np.int64)
    per_core_e = []
    for c in range(N_CORES):
        m = core == c
        order = np.lexsort((src_f[m], half[m], blk[m]))
        per_core_e.append(
            dict(src=gidx[m][order], slot=slot[m][order], w=norm[m][order])
        )
        counts[c] = np.bincount(
            blk[m] * 2 + half[m], minlength=nb * 2
        ).reshape(nb, 2)

    # common padded schedule: tiles per (block, half), maxed over cores
    tiles_bh = np.maximum(1, _ceil_div(counts.max(axis=0), 128))  # [nb, 2]
    pad_bh = tiles_bh * 128

    groups = [list(range(g, min(g + G_BLOCKS, nb))) for g in range(0, nb, G_BLOCKS)]
    slot_off = np.zeros((nb, 2), dtype=np.int64)
    call_cnt = []  # per (group, half): total padded count = gather call size
    off = 0
    for g in groups:
        for h in (0, 1):
            c0 = off
            for b in g:
                slot_off[b, h] = off
                off += pad_bh[b, h]
            call_cnt.append(off - c0)
    totslot = off
    T = totslot // 128

    ins = []
    for c in range(N_CORES):
        pc = per_core_e[c]
        idx_sl = np.zeros(totslot, dtype=np.int16)
        dst_sl = np.zeros(totslot, dtype=np.float32)
        w_sl = np.zeros(totslot, dtype=np.float32)
        starts = np.zeros((nb, 2), dtype=np.int64)
        starts.reshape(-1)[1:] = np.cumsum(counts[c].reshape(-1))[:-1]
        for b in range(nb):
            for h in (0, 1):
                n = counts[c, b, h]
                if n:
                    s0 = starts[b, h]
                    o = slot_off[b, h]
                    idx_sl[o : o + n] = pc["src"][s0 : s0 + n].astype(np.int16)
                    dst_sl[o : o + n] = pc["slot"][s0 : s0 + n]
                    w_sl[o : o + n] = pc["w"][s0 : s0 + n]

        wrapped = []
        off2 = 0
        for cc in call_cnt:
            wrapped.append(_wrap_idx(idx_sl[off2 : off2 + cc]))
            off2 += cc
        idx_w = np.concatenate(wrapped, axis=1)  # [128, totslot//16]

        col = lambda a: np.ascontiguousarray(a.reshape(T, 128).T)  # [128, T]
        xt = np.zeros((128, per_pad), dtype=np.float32)
        xt[:, :per] = np.asarray(x[c * per : (c + 1) * per], dtype=np.float32).T
        # block-diagonal dinv^2: DIAG2[p, b*128+d] = dinv2[node] iff p == d
        d2m = np.zeros((128, per_pad), dtype=np.float32)
        ar = np.arange(per)
        d2m[ar & 127, ar] = dinv2[c * per : (c + 1) * per]
        ins.append(
            dict(
                IDX=idx_w,
                DSTLOC=col(dst_sl).astype(BF16),
                WCOL=col(w_sl).astype(BF16),
                XT=xt,
                DIAG2=d2m.astype(BF16),
            )
        )

    meta = dict(
        N=N, per=per, nb=nb, per_pad=per_pad, SA=SA, T=T, totslot=totslot,
        groups=groups, call_cnt=call_cnt, tiles_bh=tiles_bh, slot_off=slot_off,
    )
    return meta, ins


def _build(meta):
    per = meta["per"]
    nb = meta["nb"]
    per_pad = meta["per_pad"]
    SA = meta["SA"]
    SB = per - SA
    T = meta["T"]
    totslot = meta["totslot"]
    groups = meta["groups"]
    call_cnt = meta["call_cnt"]
    tiles_bh = meta["tiles_bh"]
    slot_off = meta["slot_off"]

    f32 = mybir.dt.float32
    bf16 = mybir.dt.bfloat16
    i16 = mybir.dt.int16

    call_base = [sum(call_cnt[:i]) for i in range(len(call_cnt))]
    maxw16 = max(c // 16 for c in call_cnt)
    maxw128 = max(c // 128 for c in call_cnt)
    ag_gi = ((SA - 1) >> 7) // G_BLOCKS  # group whose stores complete hcurA
    # emit AG-A a few groups later so its input wait never stalls the Pool
    # queue while earlier groups' compute is still in flight
    ag_emit = min(ag_gi + 3, len(groups) - 1)

    nc = bacc.Bacc("TRN2", num_devices=N_CORES, num_swdge_queues=4,
                   dynamic_dma_scratch_size=32768)

    t_idx = nc.dram_tensor("IDX", [128, totslot // 16], i16, kind="ExternalInput")
    t_dstloc = nc.dram_tensor("DSTLOC", [128, T], bf16, kind="ExternalInput")
    t_wcol = nc.dram_tensor("WCOL", [128, T], bf16, kind="ExternalInput")
    t_xt = nc.dram_tensor("XT", [128, per_pad], f32, kind="ExternalInput")
    t_diag2 = nc.dram_tensor("DIAG2", [128, per_pad], bf16, kind="ExternalInput")
    t_W = [
        nc.dram_tensor(f"W{i}", [128, 128], f32, kind="ExternalInput") for i in (1, 2, 3)
    ]
    t_b = [
        nc.dram_tensor(f"b{i}", [128, 1], f32, kind="ExternalInput") for i in (1, 2, 3)
    ]
    t_iota_b = nc.dram_tensor("IOTAB", [128, 128], bf16, kind="ExternalInput")
    t_ident = nc.dram_tensor("IDENT", [128, 128], f32, kind="ExternalInput")
    t_identb = nc.dram_tensor("IDENTB", [128, 128], bf16, kind="ExternalInput")
    t_out = nc.dram_tensor("OUT", [per, 128], f32, kind="ExternalOutput")

    hcurA = [
        nc.dram_tensor(f"hcurA{l}", [SA, 128], bf16, kind="Internal") for l in range(3)
    ]
    hcurB = [
        nc.dram_tensor(f"hcurB{l}", [SB, 128], bf16, kind="Internal") for l in range(3)
    ]
    hfullA = [
        nc.dram_tensor(
            f"hfullA{l}", [N_CORES * SA, 128], bf16, kind="Internal",
            addr_space="Shared",
        )
        for l in range(3)
    ]
    hfullB = [
        nc.dram_tensor(
            f"hfullB{l}", [N_CORES * SB, 128], bf16, kind="Internal",
            addr_space="Shared",
        )
        for l in range(3)
    ]
    rg = [list(range(N_CORES))]

    with TileContext(nc) as tc:
        with (
            tc.tile_pool(name="persist", bufs=1) as pp,
            tc.tile_pool(name="work", bufs=2) as wp,
            tc.tile_pool(name="mp", bufs=2) as mp,
            tc.tile_pool(name="sp", bufs=2) as sp,
            tc.tile_pool(name="psg", bufs=2, space="PSUM") as psg,
            tc.tile_pool(name="psd", bufs=2, space="PSUM") as psd,
            tc.tile_pool(name="pst", bufs=2, space="PSUM") as pst,
        ):
            # ---------- persistent loads ----------
            DSTLOC = pp.tile([128, T], bf16, tag="DSTLOC")
            nc.sync.dma_start(DSTLOC[:, :], t_dstloc[:, :])
            WCOL = pp.tile([128, T], bf16, tag="WCOL")
            nc.sync.dma_start(WCOL[:, :], t_wcol[:, :])
            IOTAB = pp.tile([128, 128], bf16, tag="IOTAB")
            nc.sync.dma_start(IOTAB[:, :], t_iota_b[:, :])
            IDENT = pp.tile([128, 128], f32, tag="IDENT")
            nc.sync.dma_start(IDENT[:, :], t_ident[:, :])
            IDENTB = pp.tile([128, 128], bf16, tag="IDENTB")
            nc.sync.dma_start(IDENTB[:, :], t_identb[:, :])
            DIAG2 = pp.tile([128, per_pad], bf16, tag="DIAG2")
            nc.sync.dma_start(DIAG2[:, :], t_diag2[:, :])
            W = []
            B = []
            for i in range(3):
                Wt = pp.tile([128, 128], f32, tag=f"W{i}")
                nc.sync.dma_start(Wt[:, :], t_W[i][:, :])
                W.append(Wt)
                Bt = pp.tile([128, 1], f32, tag=f"B{i}")
                nc.sync.dma_start(Bt[:, :], t_b[i][:, :])
                B.append(Bt)

            HP = pp.tile([128, per_pad], bf16, tag="HP")
            HOUT = pp.tile([128, per_pad], f32, tag="HOUT")
            # persistent transposed rows of HP: AllGather source AND the
            # self-loop diag matmul's lhsT
            HPROWS = pp.tile([128, nb, 128], bf16, tag="HPROWS")

            def store_rows(cb, l):
                rt = HPROWS[:, cb, :]
                r0 = cb * 128
                r1 = min(per, r0 + 128)
                if r1 <= SA:
                    nc.sync.dma_start(hcurA[l][r0:r1, :], rt[0 : r1 - r0, :])
                elif r0 >= SA:
                    nc.sync.dma_start(
                        hcurB[l][r0 - SA : r1 - SA, :], rt[0 : r1 - r0, :]
                    )
                else:
                    nc.sync.dma_start(hcurA[l][r0:SA, :], rt[0 : SA - r0, :])
                    nc.sync.dma_start(
                        hcurB[l][0 : r1 - SA, :], rt[SA - r0 : r1 - r0, :]
                    )

            def all_gather(h, l):
                cur, full = (hcurA, hfullA) if h == 0 else (hcurB, hfullB)
                nc.gpsimd.collective_compute(
                    "AllGather", mybir.AluOpType.bypass,
                    ins=[cur[l][:, :]], outs=[full[l][:, :]],
                    replica_groups=rg,
                )

            def producer_group(g, l, src):
                """Transpose src chunk of HP/HOUT rows into hcur[l] stores."""
                for cb in g:
                    if src is HP:
                        pt = pst.tile([128, 128], bf16, tag="pt")
                        nc.tensor.transpose(
                            pt[:, :], src[:, cb * 128 : (cb + 1) * 128], IDENTB[:, :]
                        )
                        nc.vector.tensor_copy(HPROWS[:, cb, :], pt[:, :])
                        store_rows(cb, l)
                    else:
                        ptf = pst.tile([128, 128], f32, tag="ptf", bufs=1)
                        nc.tensor.transpose(
                            ptf[:, :], src[:, cb * 128 : (cb + 1) * 128], IDENT[:, :]
                        )
                        rf = wp.tile([128, 128], f32, tag="rowf")
                        nc.vector.tensor_copy(rf[:, :], ptf[:, :])
                        r0 = cb * 128
                        r1 = min(per, r0 + 128)
                        nc.sync.dma_start(t_out[r0:r1, :], rf[0 : r1 - r0, :])

            # ---------- layer-0 producer: HP = (x @ W1)^T, publish rows ----
            for gi, g in enumerate(groups):
                g0 = g[0] * 128
                gw = len(g) * 128
                xc = wp.tile([128, 512], f32, tag="xc")
                nc.sync.dma_start(xc[:, :gw], t_xt[:, g0 : g0 + gw])
                ph = psd.tile([128, 512], f32, tag="pd")
                nc.tensor.matmul(ph[:, :gw], W[0][:, :], xc[:, :gw], start=True, stop=True)
                nc.vector.tensor_copy(HP[:, g0 : g0 + gw], ph[:, :gw])
                producer_group(g, 0, HP)
                if gi == ag_gi:
                    all_gather(0, 0)
            all_gather(1, 0)

            # ---------- layers ----------
            for l in range(3):
                for gi, g in enumerate(groups):
                    g0 = g[0] * 128
                    gw = len(g) * 128
                    mts = {}
                    for h in (0, 1):
                        ci = 2 * gi + h
                        cnt = call_cnt[ci]
                        ntl = cnt // 128
                        woff = call_base[ci] // 16
                        idxt = wp.tile([128, maxw16], i16, tag="idx", bufs=4)
                        nc.sync.dma_start(
                            idxt[:, : cnt // 16], t_idx[:, woff : woff + cnt // 16]
                        )
                        mt = mp.tile([128, maxw128, 128], bf16, tag=f"m{h}")
                        src_tab = hfullA[l][:, :] if h == 0 else hfullB[l][:, :]
                        nc.gpsimd.dma_gather(
                            mt[:, :ntl, :], src_tab, idxt[:, : cnt // 16],
                            num_idxs=cnt, num_idxs_reg=cnt, elem_size=128,
                            single_packet=SINGLE_PACKET, queue_num=ci % 4,
                        )
                        mts[h] = mt
                    if l < 2 and gi == ag_emit:
                        all_gather(0, l + 1)
                    sws = {}
                    for h in (0, 1):
                        ci = 2 * gi + h
                        cnt = call_cnt[ci]
                        ntl = cnt // 128
                        b128 = call_base[ci] // 128
                        sw = sp.tile([128, maxw128, 128], bf16, tag=f"s{h}")
                        d_b = (
                            DSTLOC[:, b128 : b128 + ntl]
                            .unsqueeze(2)
                            .to_broadcast([128, ntl, 128])
                        )
                        i_b = IOTAB[:, :].unsqueeze(1).to_broadcast([128, ntl, 128])
                        w_b = (
                            WCOL[:, b128 : b128 + ntl]
                            .unsqueeze(2)
                            .to_broadcast([128, ntl, 128])
                        )
                        nc.vector.tensor_tensor(
                            sw[:, :ntl, :], d_b, i_b, op=mybir.AluOpType.is_equal
                        )
                        nc.vector.tensor_tensor(
                            sw[:, :ntl, :], sw[:, :ntl, :], w_b,
                            op=mybir.AluOpType.mult,
                        )
                        sws[h] = sw

                    pg = psg.tile([128, 512], f32, tag="pg")
                    for bi, b in enumerate(g):
                        seq = []
                        for h in (0, 1):
                            j0 = (slot_off[b][h] - call_base[2 * gi + h]) // 128
                            seq += [(h, j) for j in range(j0, j0 + int(tiles_bh[b][h]))]
                        for k, (h, j) in enumerate(seq):
                            nc.tensor.matmul(
                                pg[:, bi * 128 : (bi + 1) * 128],
                                mts[h][:, j, :], sws[h][:, j, :],
                                start=(k == 0), stop=False,
                            )
                        # self-loop term: out[:, d] += dinv2[d] * HP[:, d]
                        # via block-diagonal matmul (lhsT = this block's rows)
                        nc.tensor.matmul(
                            pg[:, bi * 128 : (bi + 1) * 128],
                            HPROWS[:, b, :], DIAG2[:, b * 128 : (b + 1) * 128],
                            start=False, stop=True,
                        )

                    # epilogue straight from PSUM: bias + leaky on ACT
                    if l < 2:
                        if LEAKY_VIA_PRELU:
                            nc.scalar.activation(
                                HOUT[:, g0 : g0 + gw], pg[:, :gw],
                                mybir.ActivationFunctionType.Prelu,
                                bias=B[l][:, 0:1], scale=1.0, alpha=NEG_SLOPE,
                            )
                        else:
                            t2 = wp.tile([128, 512], f32, tag="ep2")
                            nc.scalar.activation(
                                t2[:, :gw], pg[:, :gw],
                                mybir.ActivationFunctionType.Identity,
                                bias=B[l][:, 0:1], scale=1.0,
                            )
                            t3 = wp.tile([128, 512], f32, tag="ep3")
                            nc.vector.tensor_scalar_mul(
                                t3[:, :gw], t2[:, :gw], NEG_SLOPE
                            )
                            nc.vector.tensor_tensor(
                                HOUT[:, g0 : g0 + gw], t2[:, :gw], t3[:, :gw],
                                op=mybir.AluOpType.max,
                            )
                        # fused next-layer producer for this chunk
                        ph = psd.tile([128, 512], f32, tag="pd")
                        nc.tensor.matmul(
                            ph[:, :gw], W[l + 1][:, :], HOUT[:, g0 : g0 + gw],
                            start=True, stop=True,
                        )
                        nc.vector.tensor_copy(HP[:, g0 : g0 + gw], ph[:, :gw])
                        producer_group(g, l + 1, HP)
                    else:
                        nc.scalar.activation(
                            HOUT[:, g0 : g0 + gw], pg[:, :gw],
                            mybir.ActivationFunctionType.Identity,
                            bias=B[l][:, 0:1], scale=1.0,
                        )
                        producer_group(g, l, HOUT)

                if l < 2:
                    all_gather(1, l + 1)

    nc.compile()
    return nc


_CACHE = {}


def kernel(
    x,
    edge_index,
    edge_attr,
    edge_type,
    edge_type_scale,
    W1,
    b1,
    W2,
    b2,
    W3,
    b3,
):
    x = np.asarray(x)
    N = x.shape[0]
    meta, per_core = _preprocess(
        np.asarray(x), np.asarray(edge_index), np.asarray(edge_attr),
        np.asarray(edge_type), np.asarray(edge_type_scale),
    )

    key = (N, meta["T"], tuple(meta["call_cnt"]))
    if key not in _CACHE:
        _CACHE[key] = _build(meta)
    nc = _CACHE[key]

    iota_f = np.tile(np.arange(128, dtype=np.float32)[None, :], (128, 1))
    ident = np.eye(128, dtype=np.float32)
    common = dict(
        W1=np.asarray(W1, np.float32),
        W2=np.asarray(W2, np.float32),
        W3=np.asarray(W3, np.float32),
        b1=np.asarray(b1, np.float32).reshape(D, 1),
        b2=np.asarray(b2, np.float32).reshape(D, 1),
        b3=np.asarray(b3, np.float32).reshape(D, 1),
        IOTAB=iota_f.astype(BF16),
        IDENT=ident,
        IDENTB=ident.astype(BF16),
    )
    in_maps = []
    for c in range(N_CORES):
        m = dict(common)
        m.update(per_core[c])
        in_maps.append(m)

    res = run_bass_kernel_spmd(
        nc, in_maps, core_ids=list(range(N_CORES)), **_RUN_KWARGS
    )
    _LAST_RESULT.clear()
    _LAST_RESULT["exec_time_ns"] = res.exec_time_ns
    _LAST_RESULT["profile_json"] = res.profile_json
    out = np.concatenate([res.results[c]["OUT"] for c in range(N_CORES)], axis=0)
    return out.astype(np.float32)


_RUN_KWARGS = {}  # test harness can set {"trace": True, "tmpdir": ...}
_LAST_RESULT = {}


# revision 16
# speedup vs baseline: 1.4343x; 1.1508x over previous
"""3-layer GCN (message passing) on 8 TRN2 NeuronCores.

Strategy: shard destination nodes across cores (graph parallel). All edge
normalization (deg, dinv, per-edge norm = dinv_s*w*dinv_d) is precomputed on
the host. Per layer, per core:
  HP = (prev @ W)^T computed locally on the node shard (PE), rows
  transposed (PE) + stored; two AllGathers (A/B half tables, bf16) publish
  all source rows. Per group of 4 dst blocks: two SWDGE dma_gather calls
  (lo/hi halves so gather indices fit int16) pull source rows; the weighted
  one-hot S_w[e,d] = (dstloc[e]==iota[d]) * norm[e] is built on DVE with two
  broadcast-AP ops (no DRAM traffic); PE accumulates out^T = M^T @ S_w per
  block into a group PSUM bank. Epilogue adds the self-loop term
  dinv^2 * HP and bias, applies leaky-relu, and the next layer's dense
  matmul + transpose + hcur stores are fused into the same group loop so the
  next AllGather's inputs are ready the moment the layer ends.
"""

import numpy as np

import concourse.bacc as bacc
import concourse.mybir as mybir
from concourse.tile import TileContext
from concourse.bass_utils import run_bass_kernel_spmd

try:
    import ml_dtypes

    BF16 = ml_dtypes.bfloat16
except ImportError:  # pragma: no cover
    BF16 = None

N_CORES = 8
D = 128
NEG_SLOPE = 0.1
G_BLOCKS = 4  # dst blocks per matmul group / gather call pair
LEAKY_VIA_PRELU = True  # sim validation sets False (Prelu not in CoreSim)
SINGLE_PACKET = False


def _ceil_div(a, b):
    return (a + b - 1) // b


def _wrap_idx(idx):
    """[cnt] int16 -> [128, cnt//16] wrapped layout (16-partition, replicated x8)."""
    cnt = idx.shape[0]
    assert cnt % 16 == 0
    w = idx.reshape(cnt // 16, 16).T  # [16, cnt//16]
    return np.tile(w, (8, 1)).astype(np.int16)  # [128, cnt//16]


def _preprocess(x, edge_index, edge_attr, edge_type, edge_type_scale):
    """Host-side normalization + sharding/layout. Returns (meta, per-core inputs)."""
    N = x.shape[0]
    assert N % N_CORES == 0
    per = N // N_CORES
    nb = _ceil_div(per, 128)
    per_pad = nb * 128
    SA = max(16, ((per // 2) // 16) * 16)
    SB = per - SA
    assert SA * N_CORES <= 32768 and SB * N_CORES <= 32768

    src_f = np.asarray(edge_index[0], dtype=np.int64)
    dst_f = np.asarray(edge_index[1], dtype=np.int64)
    ets = np.asarray(edge_type_scale, dtype=np.float64)
    w = ets[np.asarray(edge_type, dtype=np.int64)] * np.asarray(
        edge_attr, dtype=np.float64
    )
    deg = np.bincount(dst_f, weights=w, minlength=N) + 1.0  # +1 = self loop
    dinv = 1.0 / np.sqrt(deg)
    norm = (dinv[src_f] * w * dinv[dst_f]).astype(np.float32)
    dinv2 = (dinv * dinv).astype(np.float32)

    core = dst_f // per
    ldst = dst_f - core * per
    blk = ldst >> 7
    slot = ldst & 127
    src_c = src_f // per
    src_r = src_f - src_c * per
    half = (src_r >= SA).astype(np.int64)
    gidx = np.where(half == 0, src_c * SA + src_r, src_c * SB + (src_r - SA))

    counts = np.zeros((N_CORES, nb, 2), dtype=np.int64)
    per_core_e = []
    for c in range(N_CORES):
        m = core == c
        order = np.lexsort((src_f[m], half[m], blk[m]))
        per_core_e.append(
            dict(src=gidx[m][order], slot=slot[m][order], w=norm[m][order])
        )
        counts[c] = np.bincount(
            blk[m] * 2 + half[m], minlength=nb * 2
        ).reshape(nb, 2)

    # common padded schedule: tiles per (block, half), maxed over cores
    tiles_bh = np.maximum(1, _ceil_div(counts.max(axis=0), 128))  # [nb, 2]
    pad_bh = tiles_bh * 128

    groups = [list(range(g, min(g + G_BLOCKS, nb))) for g in range(0, nb, G_BLOCKS)]
    slot_off = np.zeros((nb, 2), dtype=np.int64)
    call_cnt = []  # per (group, half): total padded count = gather call size
    off = 0
    for g in groups:
        for h in (0, 1):
            c0 = off
            for b in g:
                slot_off[b, h] = off
                off += pad_bh[b, h]
            call_cnt.append(off - c0)
    totslot = off
    T = totslot // 128

    ins = []
    for c in range(N_CORES):
        pc = per_core_e[c]
        idx_sl = np.zeros(totslot, dtype=np.int16)
        dst_sl = np.zeros(totslot, dtype=np.float32)
        w_sl = np.zeros(totslot, dtype=np.float32)
        starts = np.zeros((nb, 2), dtype=np.int64)
        starts.reshape(-1)[1:] = np.cumsum(counts[c].reshape(-1))[:-1]
        for b in range(nb):
            for h in (0, 1):
                n = counts[c, b, h]
                if n:
                    s0 = starts[b, h]
                    o = slot_off[b, h]
                    idx_sl[o : o + n] = pc["src"][s0 : s0 + n].astype(np.int16)
                    dst_sl[o : o + n] = pc["slot"][s0 : s0 + n]
                    w_sl[o : o + n] = pc["w"][s0 : s0 + n]

        wrapped = []
        off2 = 0
        for cc in call_cnt:
            wrapped.append(_wrap_idx(idx_sl[off2 : off2 + cc]))
            off2 += cc
        idx_w = np.concatenate(wrapped, axis=1)  # [128, totslot//16]

        col = lambda a: np.ascontiguousarray(a.reshape(T, 128).T)  # [128, T]
        xt = np.zeros((128, per_pad), dtype=np.float32)
        xt[:, :per] = np.asarray(x[c * per : (c + 1) * per], dtype=np.float32).T
        # block-diagonal dinv^2: DIAG2[p, b*128+d] = dinv2[node] iff p == d
        d2m = np.zeros((128, per_pad), dtype=np.float32)
        ar = np.arange(per)
        d2m[ar & 127, ar] = dinv2[c * per : (c + 1) * per]
        ins.append(
            dict(
                IDX=idx_w,
                DSTLOC=col(dst_sl).astype(BF16),
                WCOL=col(w_sl).astype(BF16),
                XT=xt,
                DIAG2=d2m.astype(BF16),
            )
        )

    meta = dict(
        N=N, per=per, nb=nb, per_pad=per_pad, SA=SA, T=T, totslot=totslot,
        groups=groups, call_cnt=call_cnt, tiles_bh=tiles_bh, slot_off=slot_off,
    )
    return meta, ins


def _build(meta):
    per = meta["per"]
    nb = meta["nb"]
    per_pad = meta["per_pad"]
    SA = meta["SA"]
    SB = per - SA
    T = meta["T"]
    totslot = meta["totslot"]
    groups = meta["groups"]
    call_cnt = meta["call_cnt"]
    tiles_bh = meta["tiles_bh"]
    slot_off = meta["slot_off"]

    f32 = mybir.dt.float32
    bf16 = mybir.dt.bfloat16
    i16 = mybir.dt.int16

    call_base = [sum(call_cnt[:i]) for i in range(len(call_cnt))]
    maxw16 = max(c // 16 for c in call_cnt)
    maxw128 = max(c // 128 for c in call_cnt)
    ag_gi = ((SA - 1) >> 7) // G_BLOCKS  # group whose stores complete hcurA
    # emit AG-A a few groups later so its input wait never stalls the Pool
    # queue while earlier groups' compute is still in flight
    ag_emit = min(ag_gi + 3, len(groups) - 1)

    nc = bacc.Bacc("TRN2", num_devices=N_CORES, num_swdge_queues=4,
                   dynamic_dma_scratch_size=32768)

    t_idx = nc.dram_tensor("IDX", [128, totslot // 16], i16, kind="ExternalInput")
    t_dstloc = nc.dram_tensor("DSTLOC", [128, T], bf16, kind="ExternalInput")
    t_wcol = nc.dram_tensor("WCOL", [128, T], bf16, kind="ExternalInput")
    t_xt = nc.dram_tensor("XT", [128, per_pad], f32, kind="ExternalInput")
    t_diag2 = nc.dram_tensor("DIAG2", [128, per_pad], bf16, kind="ExternalInput")
    t_W = [
        nc.dram_tensor(f"W{i}", [128, 128], f32, kind="ExternalInput") for i in (1, 2, 3)
    ]
    t_b = [
        nc.dram_tensor(f"b{i}", [128, 1], f32, kind="ExternalInput") for i in (1, 2, 3)
    ]
    t_iota_b = nc.dram_tensor("IOTAB", [128, 128], bf16, kind="ExternalInput")
    t_ident = nc.dram_tensor("IDENT", [128, 128], f32, kind="ExternalInput")
    t_identb = nc.dram_tensor("IDENTB", [128, 128], bf16, kind="ExternalInput")
    t_out = nc.dram_tensor("OUT", [per, 128], f32, kind="ExternalOutput")

    hcurA = [
        nc.dram_tensor(f"hcurA{l}", [SA, 128], bf16, kind="Internal") for l in range(3)
    ]
    hcurB = [
        nc.dram_tensor(f"hcurB{l}", [SB, 128], bf16, kind="Internal") for l in range(3)
    ]
    hfullA = [
        nc.dram_tensor(
            f"hfullA{l}", [N_CORES * SA, 128], bf16, kind="Internal",
            addr_space="Shared",
        )
        for l in range(3)
    ]
    hfullB = [
        nc.dram_tensor(
            f"hfullB{l}", [N_CORES * SB, 128], bf16, kind="Internal",
            addr_space="Shared",
        )
        for l in range(3)
    ]
    rg = [list(range(N_CORES))]

    with TileContext(nc) as tc:
        with (
            tc.tile_pool(name="persist", bufs=1) as pp,
            tc.tile_pool(name="work", bufs=2) as wp,
            tc.tile_pool(name="mp", bufs=2) as mp,
            tc.tile_pool(name="sp", bufs=2) as sp,
            tc.tile_pool(name="psg", bufs=2, space="PSUM") as psg,
            tc.tile_pool(name="psd", bufs=2, space="PSUM") as psd,
            tc.tile_pool(name="pst", bufs=2, space="PSUM") as pst,
        ):
            # ---------- persistent loads ----------
            DSTLOC = pp.tile([128, T], bf16, tag="DSTLOC")
            nc.sync.dma_start(DSTLOC[:, :], t_dstloc[:, :])
            WCOL = pp.tile([128, T], bf16, tag="WCOL")
            nc.sync.dma_start(WCOL[:, :], t_wcol[:, :])
            IOTAB = pp.tile([128, 128], bf16, tag="IOTAB")
            nc.sync.dma_start(IOTAB[:, :], t_iota_b[:, :])
            IDENT = pp.tile([128, 128], f32, tag="IDENT")
            nc.sync.dma_start(IDENT[:, :], t_ident[:, :])
            IDENTB = pp.tile([128, 128], bf16, tag="IDENTB")
            nc.sync.dma_start(IDENTB[:, :], t_identb[:, :])
            DIAG2 = pp.tile([128, per_pad], bf16, tag="DIAG2")
            nc.sync.dma_start(DIAG2[:, :], t_diag2[:, :])
            W = []
            B = []
            for i in range(3):
                Wt = pp.tile([128, 128], f32, tag=f"W{i}")
                nc.sync.dma_start(Wt[:, :], t_W[i][:, :])
                W.append(Wt)
                Bt = pp.tile([128, 1], f32, tag=f"B{i}")
                nc.sync.dma_start(Bt[:, :], t_b[i][:, :])
                B.append(Bt)

            HOUT = pp.tile([128, per_pad], f32, tag="HOUT")
            # persistent transposed rows of HP: AllGather source AND the
            # self-loop diag matmul's lhsT
            HPROWS = pp.tile([128, nb, 128], bf16, tag="HPROWS")

            def store_rows(cb, l):
                rt = HPROWS[:, cb, :]
                r0 = cb * 128
                r1 = min(per, r0 + 128)
                if r1 <= SA:
                    nc.sync.dma_start(hcurA[l][r0:r1, :], rt[0 : r1 - r0, :])
                elif r0 >= SA:
                    nc.sync.dma_start(
                        hcurB[l][r0 - SA : r1 - SA, :], rt[0 : r1 - r0, :]
                    )
                else:
                    nc.sync.dma_start(hcurA[l][r0:SA, :], rt[0 : SA - r0, :])
                    nc.sync.dma_start(
                        hcurB[l][0 : r1 - SA, :], rt[SA - r0 : r1 - r0, :]
                    )

            def all_gather(h, l):
                cur, full = (hcurA, hfullA) if h == 0 else (hcurB, hfullB)
                nc.gpsimd.collective_compute(
                    "AllGather", mybir.AluOpType.bypass,
                    ins=[cur[l][:, :]], outs=[full[l][:, :]],
                    replica_groups=rg,
                )

            def act_copy(dst_ap, src_ap):
                # copies ride the idle Scalar engine so the DVE queue stays
                # pure one-hot builds (no head-of-line blocking)
                nc.scalar.activation(
                    dst_ap, src_ap, mybir.ActivationFunctionType.Identity,
                    bias=0.0, scale=1.0,
                )

            def producer_group(g, l, hpf, g0, final=False):
                """Transpose rows of the group chunk into hcur[l] / OUT."""
                for cb in g:
                    co = cb * 128 - g0
                    if not final:
                        pt = pst.tile([128, 128], f32, tag="pt")
                        nc.tensor.transpose(
                            pt[:, :], hpf[:, co : co + 128], IDENT[:, :]
                        )
                        act_copy(HPROWS[:, cb, :], pt[:, :])
                        store_rows(cb, l)
                    else:
                        ptf = pst.tile([128, 128], f32, tag="ptf", bufs=1)
                        nc.tensor.transpose(
                            ptf[:, :], hpf[:, co : co + 128], IDENT[:, :]
                        )
                        rf = wp.tile([128, 128], f32, tag="rowf")
                        act_copy(rf[:, :], ptf[:, :])
                        r0 = cb * 128
                        r1 = min(per, r0 + 128)
                        nc.sync.dma_start(t_out[r0:r1, :], rf[0 : r1 - r0, :])

            # ---------- layer-0 producer: HP = (x @ W1)^T, publish rows ----
            for gi, g in enumerate(groups):
                g0 = g[0] * 128
                gw = len(g) * 128
                xc = wp.tile([128, 512], f32, tag="xc")
                nc.sync.dma_start(xc[:, :gw], t_xt[:, g0 : g0 + gw])
                ph = psd.tile([128, 512], f32, tag="pd")
                nc.tensor.matmul(ph[:, :gw], W[0][:, :], xc[:, :gw], start=True, stop=True)
                hpf = wp.tile([128, 512], f32, tag="hpf")
                act_copy(hpf[:, :gw], ph[:, :gw])
                producer_group(g, 0, hpf, g0)
                if gi == ag_gi:
                    all_gather(0, 0)
            all_gather(1, 0)

            # ---------- layers ----------
            for l in range(3):
                for gi, g in enumerate(groups):
                    g0 = g[0] * 128
                    gw = len(g) * 128
                    mts = {}
                    for h in (0, 1):
                        ci = 2 * gi + h
                        cnt = call_cnt[ci]
                        ntl = cnt // 128
                        woff = call_base[ci] // 16
                        idxt = wp.tile([128, maxw16], i16, tag="idx", bufs=4)
                        nc.sync.dma_start(
                            idxt[:, : cnt // 16], t_idx[:, woff : woff + cnt // 16]
                        )
                        mt = mp.tile([128, maxw128, 128], bf16, tag=f"m{h}", bufs=3)
                        src_tab = hfullA[l][:, :] if h == 0 else hfullB[l][:, :]
                        nc.gpsimd.dma_gather(
                            mt[:, :ntl, :], src_tab, idxt[:, : cnt // 16],
                            num_idxs=cnt, num_idxs_reg=cnt, elem_size=128,
                            single_packet=SINGLE_PACKET, queue_num=ci % 4,
                        )
                        mts[h] = mt
                    if l < 2 and gi == ag_emit:
                        all_gather(0, l + 1)
                    sws = {}
                    for h in (0, 1):
                        ci = 2 * gi + h
                        cnt = call_cnt[ci]
                        ntl = cnt // 128
                        b128 = call_base[ci] // 128
                        sw = sp.tile([128, maxw128, 128], bf16, tag=f"s{h}")
                        d_b = (
                            DSTLOC[:, b128 : b128 + ntl]
                            .unsqueeze(2)
                            .to_broadcast([128, ntl, 128])
                        )
                        i_b = IOTAB[:, :].unsqueeze(1).to_broadcast([128, ntl, 128])
                        w_b = (
                            WCOL[:, b128 : b128 + ntl]
                            .unsqueeze(2)
                            .to_broadcast([128, ntl, 128])
                        )
                        nc.vector.tensor_tensor(
                            sw[:, :ntl, :], d_b, i_b, op=mybir.AluOpType.is_equal
                        )
                        nc.vector.tensor_tensor(
                            sw[:, :ntl, :], sw[:, :ntl, :], w_b,
                            op=mybir.AluOpType.mult,
                        )
                        sws[h] = sw

                    pg = psg.tile([128, 512], f32, tag="pg")
                    for bi, b in enumerate(g):
                        seq = []
                        for h in (0, 1):
                            j0 = (slot_off[b][h] - call_base[2 * gi + h]) // 128
                            seq += [(h, j) for j in range(j0, j0 + int(tiles_bh[b][h]))]
                        for k, (h, j) in enumerate(seq):
                            nc.tensor.matmul(
                                pg[:, bi * 128 : (bi + 1) * 128],
                                mts[h][:, j, :], sws[h][:, j, :],
                                start=(k == 0), stop=False,
                            )
                        # self-loop term: out[:, d] += dinv2[d] * HP[:, d]
                        # via block-diagonal matmul (lhsT = this block's rows)
                        nc.tensor.matmul(
                            pg[:, bi * 128 : (bi + 1) * 128],
                            HPROWS[:, b, :], DIAG2[:, b * 128 : (b + 1) * 128],
                            start=False, stop=True,
                        )

                    # epilogue straight from PSUM: bias + leaky on ACT
                    if l < 2:
                        if LEAKY_VIA_PRELU:
                            nc.scalar.activation(
                                HOUT[:, g0 : g0 + gw], pg[:, :gw],
                                mybir.ActivationFunctionType.Prelu,
                                bias=B[l][:, 0:1], scale=1.0, alpha=NEG_SLOPE,
                            )
                        else:
                            t2 = wp.tile([128, 512], f32, tag="ep2")
                            nc.scalar.activation(
                                t2[:, :gw], pg[:, :gw],
                                mybir.ActivationFunctionType.Identity,
                                bias=B[l][:, 0:1], scale=1.0,
                            )
                            t3 = wp.tile([128, 512], f32, tag="ep3")
                            nc.vector.tensor_scalar_mul(
                                t3[:, :gw], t2[:, :gw], NEG_SLOPE
                            )
                            nc.vector.tensor_tensor(
                                HOUT[:, g0 : g0 + gw], t2[:, :gw], t3[:, :gw],
                                op=mybir.AluOpType.max,
                            )
                        # fused next-layer producer for this chunk
                        ph = psd.tile([128, 512], f32, tag="pd")
                        nc.tensor.matmul(
                            ph[:, :gw], W[l + 1][:, :], HOUT[:, g0 : g0 + gw],
                            start=True, stop=True,
                        )
                        hpf = wp.tile([128, 512], f32, tag="hpf")
                        act_copy(hpf[:, :gw], ph[:, :gw])
                        producer_group(g, l + 1, hpf, g0)
                    else:
                        nc.scalar.activation(
                            HOUT[:, g0 : g0 + gw], pg[:, :gw],
                            mybir.ActivationFunctionType.Identity,
                            bias=B[l][:, 0:1], scale=1.0,
                        )
                        producer_group(g, l, HOUT[:, g0 : g0 + gw], g0, final=True)

                if l < 2:
                    all_gather(1, l + 1)

    nc.compile()
    return nc


_CACHE = {}


def kernel(
    x,
    edge_index,
    edge_attr,
    edge_type,
    edge_type_scale,
    W1,
    b1,
    W2,
    b2,
    W3,
    b3,
):
    x = np.asarray(x)
    N = x.shape[0]
    meta, per_core = _preprocess(
        np.asarray(x), np.asarray(edge_index), np.asarray(edge_attr),
        np.asarray(edge_type), np.asarray(edge_type_scale),
    )

    key = (N, meta["T"], tuple(meta["call_cnt"]))
    if key not in _CACHE:
        _CACHE[key] = _build(meta)
    nc = _CACHE[key]

    iota_f = np.tile(np.arange(128, dtype=np.float32)[None, :], (128, 1))
    ident = np.eye(128, dtype=np.float32)
    common = dict(
        W1=np.asarray(W1, np.float32),
        W2=np.asarray(W2, np.float32),
        W3=np.asarray(W3, np.float32),
        b1=np.asarray(b1, np.float32).reshape(D, 1),
        b2=np.asarray(b2, np.float32).reshape(D, 1),
        b3=np.asarray(b3, np.float32).reshape(D, 1),
        IOTAB=iota_f.astype(BF16),
        IDENT=ident,
        IDENTB=ident.astype(BF16),
    )
    in_maps = []
    for c in range(N_CORES):
        m = dict(common)
        m.update(per_core[c])
        in_maps.append(m)

    res = run_bass_kernel_spmd(
        nc, in_maps, core_ids=list(range(N_CORES)), **_RUN_KWARGS
    )
    _LAST_RESULT.clear()
    _LAST_RESULT["exec_time_ns"] = res.exec_time_ns
    _LAST_RESULT["profile_json"] = res.profile_json
    out = np.concatenate([res.results[c]["OUT"] for c in range(N_CORES)], axis=0)
    return out.astype(np.float32)


_RUN_KWARGS = {}  # test harness can set {"trace": True, "tmpdir": ...}
_LAST_RESULT = {}


# revision 18
# speedup vs baseline: 1.8178x; 1.2673x over previous
"""3-layer GCN (message passing) on 8 TRN2 NeuronCores.

Strategy: shard destination nodes across cores (graph parallel). All edge
normalization (deg, dinv, per-edge norm = dinv_s*w*dinv_d) is precomputed on
the host. Per layer, per core:
  HP = (prev @ W)^T computed locally on the node shard (PE), rows
  transposed (PE) + stored; two AllGathers (A/B half tables, bf16) publish
  all source rows. Per group of 4 dst blocks: two SWDGE dma_gather calls
  (lo/hi halves so gather indices fit int16) pull source rows; the weighted
  one-hot S_w[e,d] = (dstloc[e]==iota[d]) * norm[e] is built on DVE with two
  broadcast-AP ops (no DRAM traffic); PE accumulates out^T = M^T @ S_w per
  block into a group PSUM bank. Epilogue adds the self-loop term
  dinv^2 * HP and bias, applies leaky-relu, and the next layer's dense
  matmul + transpose + hcur stores are fused into the same group loop so the
  next AllGather's inputs are ready the moment the layer ends.
"""

import numpy as np

import concourse.bacc as bacc
import concourse.mybir as mybir
from concourse.tile import TileContext
from concourse.bass_utils import run_bass_kernel_spmd

try:
    import ml_dtypes

    BF16 = ml_dtypes.bfloat16
except ImportError:  # pragma: no cover
    BF16 = None

N_CORES = 8
D = 128
NEG_SLOPE = 0.1
G_BLOCKS = 4  # dst blocks per matmul group / gather call pair
LEAKY_VIA_PRELU = True  # sim validation sets False (Prelu not in CoreSim)
SINGLE_PACKET = False


def _ceil_div(a, b):
    return (a + b - 1) // b


def _wrap_idx(idx):
    """[cnt] int16 -> [128, cnt//16] wrapped layout (16-partition, replicated x8)."""
    cnt = idx.shape[0]
    assert cnt % 16 == 0
    w = idx.reshape(cnt // 16, 16).T  # [16, cnt//16]
    return np.tile(w, (8, 1)).astype(np.int16)  # [128, cnt//16]


def _preprocess(x, edge_index, edge_attr, edge_type, edge_type_scale):
    """Host-side normalization + sharding/layout. Returns (meta, per-core inputs)."""
    N = x.shape[0]
    assert N % N_CORES == 0
    per = N // N_CORES
    nb = _ceil_div(per, 128)
    per_pad = nb * 128
    # A half = first NBA blocks (aligned to G_BLOCKS), B half = rest.
    NBA = ((nb // 2) // G_BLOCKS) * G_BLOCKS
    NBB = nb - NBA
    SA = NBA * 128
    SB = per - SA
    assert SA * N_CORES <= 32768 and (NBB * 128) * N_CORES <= 32768

    src_f = np.asarray(edge_index[0], dtype=np.int64)
    dst_f = np.asarray(edge_index[1], dtype=np.int64)
    ets = np.asarray(edge_type_scale, dtype=np.float64)
    w = ets[np.asarray(edge_type, dtype=np.int64)] * np.asarray(
        edge_attr, dtype=np.float64
    )
    deg = np.bincount(dst_f, weights=w, minlength=N) + 1.0  # +1 = self loop
    dinv = 1.0 / np.sqrt(deg)
    norm = (dinv[src_f] * w * dinv[dst_f]).astype(np.float32)
    dinv2 = (dinv * dinv).astype(np.float32)

    core = dst_f // per
    ldst = dst_f - core * per
    blk = ldst >> 7
    slot = ldst & 127
    src_c = src_f // per
    src_r = src_f - src_c * per
    src_p = src_r & 127   # partition (slot within block)
    src_b = src_r >> 7    # block
    half = (src_b >= NBA).astype(np.int64)
    # p-major table layout: all of partition p's nodes are consecutive rows
    gidx = np.where(
        half == 0,
        src_c * SA + src_p * NBA + src_b,
        src_c * (NBB * 128) + src_p * NBB + (src_b - NBA),
    )

    counts = np.zeros((N_CORES, nb, 2), dtype=np.int64)
    per_core_e = []
    for c in range(N_CORES):
        m = core == c
        order = np.lexsort((src_f[m], half[m], blk[m]))
        per_core_e.append(
            dict(src=gidx[m][order], slot=slot[m][order], w=norm[m][order])
        )
        counts[c] = np.bincount(
            blk[m] * 2 + half[m], minlength=nb * 2
        ).reshape(nb, 2)

    # common padded schedule: tiles per (block, half), maxed over cores
    tiles_bh = np.maximum(1, _ceil_div(counts.max(axis=0), 128))  # [nb, 2]
    pad_bh = tiles_bh * 128

    groups = [list(range(g, min(g + G_BLOCKS, nb))) for g in range(0, nb, G_BLOCKS)]
    slot_off = np.zeros((nb, 2), dtype=np.int64)
    call_cnt = []  # per (group, half): total padded count = gather call size
    off = 0
    for g in groups:
        for h in (0, 1):
            c0 = off
            for b in g:
                slot_off[b, h] = off
                off += pad_bh[b, h]
            call_cnt.append(off - c0)
    totslot = off
    T = totslot // 128

    ins = []
    for c in range(N_CORES):
        pc = per_core_e[c]
        idx_sl = np.zeros(totslot, dtype=np.int16)
        dst_sl = np.zeros(totslot, dtype=np.float32)
        w_sl = np.zeros(totslot, dtype=np.float32)
        starts = np.zeros((nb, 2), dtype=np.int64)
        starts.reshape(-1)[1:] = np.cumsum(counts[c].reshape(-1))[:-1]
        for b in range(nb):
            for h in (0, 1):
                n = counts[c, b, h]
                if n:
                    s0 = starts[b, h]
                    o = slot_off[b, h]
                    idx_sl[o : o + n] = pc["src"][s0 : s0 + n].astype(np.int16)
                    dst_sl[o : o + n] = pc["slot"][s0 : s0 + n]
                    w_sl[o : o + n] = pc["w"][s0 : s0 + n]

        wrapped = []
        off2 = 0
        for cc in call_cnt:
            wrapped.append(_wrap_idx(idx_sl[off2 : off2 + cc]))
            off2 += cc
        idx_w = np.concatenate(wrapped, axis=1)  # [128, totslot//16]

        col = lambda a: np.ascontiguousarray(a.reshape(T, 128).T)  # [128, T]
        xt = np.zeros((128, per_pad), dtype=np.float32)
        xt[:, :per] = np.asarray(x[c * per : (c + 1) * per], dtype=np.float32).T
        # block-diagonal dinv^2: DIAG2[p, b*128+d] = dinv2[node] iff p == d
        d2m = np.zeros((128, per_pad), dtype=np.float32)
        ar = np.arange(per)
        d2m[ar & 127, ar] = dinv2[c * per : (c + 1) * per]
        ins.append(
            dict(
                IDX=idx_w,
                DSTLOC=col(dst_sl).astype(BF16),
                WCOL=col(w_sl).astype(BF16),
                XT=xt,
                DIAG2=d2m.astype(BF16),
            )
        )

    meta = dict(
        N=N, per=per, nb=nb, per_pad=per_pad, SA=SA, NBA=NBA, NBB=NBB, T=T,
        totslot=totslot, groups=groups, call_cnt=call_cnt, tiles_bh=tiles_bh,
        slot_off=slot_off,
    )
    return meta, ins


def _build(meta):
    per = meta["per"]
    nb = meta["nb"]
    per_pad = meta["per_pad"]
    SA = meta["SA"]
    NBA = meta["NBA"]
    NBB = meta["NBB"]
    SBP = NBB * 128  # B-half table rows (incl. last-block padding)
    T = meta["T"]
    totslot = meta["totslot"]
    groups = meta["groups"]
    call_cnt = meta["call_cnt"]
    tiles_bh = meta["tiles_bh"]
    slot_off = meta["slot_off"]

    f32 = mybir.dt.float32
    bf16 = mybir.dt.bfloat16
    i16 = mybir.dt.int16

    call_base = [sum(call_cnt[:i]) for i in range(len(call_cnt))]
    maxw16 = max(c // 16 for c in call_cnt)
    maxw128 = max(c // 128 for c in call_cnt)
    ag_gi = ((SA - 1) >> 7) // G_BLOCKS  # group whose stores complete hcurA
    # emit AG-A a few groups later so its input wait never stalls the Pool
    # queue while earlier groups' compute is still in flight
    ag_emit = min(ag_gi + 3, len(groups) - 1)

    nc = bacc.Bacc("TRN2", num_devices=N_CORES, num_swdge_queues=4,
                   dynamic_dma_scratch_size=32768)

    t_idx = nc.dram_tensor("IDX", [128, totslot // 16], i16, kind="ExternalInput")
    t_dstloc = nc.dram_tensor("DSTLOC", [128, T], bf16, kind="ExternalInput")
    t_wcol = nc.dram_tensor("WCOL", [128, T], bf16, kind="ExternalInput")
    t_xt = nc.dram_tensor("XT", [128, per_pad], f32, kind="ExternalInput")
    t_diag2 = nc.dram_tensor("DIAG2", [128, per_pad], bf16, kind="ExternalInput")
    t_W = [
        nc.dram_tensor(f"W{i}", [128, 128], f32, kind="ExternalInput") for i in (1, 2, 3)
    ]
    t_b = [
        nc.dram_tensor(f"b{i}", [128, 1], f32, kind="ExternalInput") for i in (1, 2, 3)
    ]
    t_iota_b = nc.dram_tensor("IOTAB", [128, 128], bf16, kind="ExternalInput")
    t_ident = nc.dram_tensor("IDENT", [128, 128], f32, kind="ExternalInput")
    t_identb = nc.dram_tensor("IDENTB", [128, 128], bf16, kind="ExternalInput")
    t_out = nc.dram_tensor("OUT", [per, 128], f32, kind="ExternalOutput")

    hcurA = [
        nc.dram_tensor(f"hcurA{l}", [SA, 128], bf16, kind="Internal") for l in range(3)
    ]
    hcurB = [
        nc.dram_tensor(f"hcurB{l}", [SBP, 128], bf16, kind="Internal") for l in range(3)
    ]
    hfullA = [
        nc.dram_tensor(
            f"hfullA{l}", [N_CORES * SA, 128], bf16, kind="Internal",
            addr_space="Shared",
        )
        for l in range(3)
    ]
    hfullB = [
        nc.dram_tensor(
            f"hfullB{l}", [N_CORES * SBP, 128], bf16, kind="Internal",
            addr_space="Shared",
        )
        for l in range(3)
    ]
    rg = [list(range(N_CORES))]

    with TileContext(nc) as tc:
        with (
            tc.tile_pool(name="persist", bufs=1) as pp,
            tc.tile_pool(name="work", bufs=2) as wp,
            tc.tile_pool(name="mp", bufs=2) as mp,
            tc.tile_pool(name="sp", bufs=2) as sp,
            tc.tile_pool(name="psg", bufs=2, space="PSUM") as psg,
            tc.tile_pool(name="psd", bufs=2, space="PSUM") as psd,
            tc.tile_pool(name="pst", bufs=2, space="PSUM") as pst,
        ):
            # ---------- persistent loads ----------
            DSTLOC = pp.tile([128, T], bf16, tag="DSTLOC")
            nc.sync.dma_start(DSTLOC[:, :], t_dstloc[:, :])
            IDXS = pp.tile([128, totslot // 16], i16, tag="IDXS")
            nc.sync.dma_start(IDXS[:, :], t_idx[:, :])
            WCOL = pp.tile([128, T], bf16, tag="WCOL")
            nc.sync.dma_start(WCOL[:, :], t_wcol[:, :])
            IOTAB = pp.tile([128, 128], bf16, tag="IOTAB")
            nc.sync.dma_start(IOTAB[:, :], t_iota_b[:, :])
            IDENT = pp.tile([128, 128], f32, tag="IDENT")
            nc.sync.dma_start(IDENT[:, :], t_ident[:, :])
            IDENTB = pp.tile([128, 128], bf16, tag="IDENTB")
            nc.sync.dma_start(IDENTB[:, :], t_identb[:, :])
            DIAG2 = pp.tile([128, per_pad], bf16, tag="DIAG2")
            nc.sync.dma_start(DIAG2[:, :], t_diag2[:, :])
            W = []
            B = []
            for i in range(3):
                Wt = pp.tile([128, 128], f32, tag=f"W{i}")
                nc.sync.dma_start(Wt[:, :], t_W[i][:, :])
                W.append(Wt)
                Bt = pp.tile([128, 1], f32, tag=f"B{i}")
                nc.sync.dma_start(Bt[:, :], t_b[i][:, :])
                B.append(Bt)

            HOUT = pp.tile([128, per_pad], f32, tag="HOUT")
            # persistent transposed rows of HP: AllGather source AND the
            # self-loop diag matmul's lhsT
            HPROWS = pp.tile([128, nb, 128], bf16, tag="HPROWS")

            def store_group(g, l):
                # one batched store per group: p-major table rows mean each
                # partition writes one contiguous len(g)*256B segment
                b0 = g[0]
                if b0 < NBA:
                    view = hcurA[l][:, :].rearrange("(p b) d -> p b d", p=128)
                    nc.sync.dma_start(
                        view[:, b0 : b0 + len(g), :], HPROWS[:, b0 : b0 + len(g), :]
                    )
                else:
                    view = hcurB[l][:, :].rearrange("(p b) d -> p b d", p=128)
                    nc.sync.dma_start(
                        view[:, b0 - NBA : b0 - NBA + len(g), :],
                        HPROWS[:, b0 : b0 + len(g), :],
                    )

            def all_gather(h, l):
                cur, full = (hcurA, hfullA) if h == 0 else (hcurB, hfullB)
                nc.gpsimd.collective_compute(
                    "AllGather", mybir.AluOpType.bypass,
                    ins=[cur[l][:, :]], outs=[full[l][:, :]],
                    replica_groups=rg,
                )

            def act_copy(dst_ap, src_ap):
                # copies ride the idle Scalar engine so the DVE queue stays
                # pure one-hot builds (no head-of-line blocking)
                nc.scalar.activation(
                    dst_ap, src_ap, mybir.ActivationFunctionType.Identity,
                    bias=0.0, scale=1.0,
                )

            def producer_group(g, l, hpf, g0, final=False):
                """Transpose rows of the group chunk into hcur[l] / OUT."""
                for cb in g:
                    co = cb * 128 - g0
                    if not final:
                        pt = pst.tile([128, 128], f32, tag="pt")
                        nc.tensor.transpose(
                            pt[:, :], hpf[:, co : co + 128], IDENT[:, :]
                        )
                        act_copy(HPROWS[:, cb, :], pt[:, :])
                    else:
                        ptf = pst.tile([128, 128], f32, tag="ptf", bufs=1)
                        nc.tensor.transpose(
                            ptf[:, :], hpf[:, co : co + 128], IDENT[:, :]
                        )
                        rf = wp.tile([128, 128], f32, tag="rowf")
                        act_copy(rf[:, :], ptf[:, :])
                        r0 = cb * 128
                        r1 = min(per, r0 + 128)
                        nc.sync.dma_start(t_out[r0:r1, :], rf[0 : r1 - r0, :])
                if not final:
                    store_group(g, l)

            # ---------- layer-0 producer: HP = (x @ W1)^T, publish rows ----
            for gi, g in enumerate(groups):
                g0 = g[0] * 128
                gw = len(g) * 128
                xc = wp.tile([128, 512], f32, tag="xc")
                nc.sync.dma_start(xc[:, :gw], t_xt[:, g0 : g0 + gw])
                ph = psd.tile([128, 512], f32, tag="pd")
                nc.tensor.matmul(ph[:, :gw], W[0][:, :], xc[:, :gw], start=True, stop=True)
                hpf = wp.tile([128, 512], f32, tag="hpf")
                act_copy(hpf[:, :gw], ph[:, :gw])
                producer_group(g, 0, hpf, g0)
                if gi == ag_gi:
                    all_gather(0, 0)
            all_gather(1, 0)

            # ---------- layers ----------
            for l in range(3):
                for gi, g in enumerate(groups):
                    g0 = g[0] * 128
                    gw = len(g) * 128
                    mts = {}
                    for h in (0, 1):
                        ci = 2 * gi + h
                        cnt = call_cnt[ci]
                        ntl = cnt // 128
                        woff = call_base[ci] // 16
                        mt = mp.tile([128, maxw128, 128], bf16, tag=f"m{h}", bufs=3)
                        src_tab = hfullA[l][:, :] if h == 0 else hfullB[l][:, :]
                        nc.gpsimd.dma_gather(
                            mt[:, :ntl, :], src_tab,
                            IDXS[:, woff : woff + cnt // 16],
                            num_idxs=cnt, num_idxs_reg=cnt, elem_size=128,
                            single_packet=SINGLE_PACKET, queue_num=ci % 4,
                        )
                        mts[h] = mt
                    if l < 2 and gi == ag_emit:
                        all_gather(0, l + 1)
                    sws = {}
                    for h in (0, 1):
                        ci = 2 * gi + h
                        cnt = call_cnt[ci]
                        ntl = cnt // 128
                        b128 = call_base[ci] // 128
                        sw = sp.tile([128, maxw128, 128], bf16, tag=f"s{h}")
                        d_b = (
                            DSTLOC[:, b128 : b128 + ntl]
                            .unsqueeze(2)
                            .to_broadcast([128, ntl, 128])
                        )
                        i_b = IOTAB[:, :].unsqueeze(1).to_broadcast([128, ntl, 128])
                        w_b = (
                            WCOL[:, b128 : b128 + ntl]
                            .unsqueeze(2)
                            .to_broadcast([128, ntl, 128])
                        )
                        nc.vector.tensor_tensor(
                            sw[:, :ntl, :], d_b, i_b, op=mybir.AluOpType.is_equal
                        )
                        nc.vector.tensor_tensor(
                            sw[:, :ntl, :], sw[:, :ntl, :], w_b,
                            op=mybir.AluOpType.mult,
                        )
                        sws[h] = sw

                    pg = psg.tile([128, 512], f32, tag="pg")
                    for bi, b in enumerate(g):
                        seq = []
                        for h in (0, 1):
                            j0 = (slot_off[b][h] - call_base[2 * gi + h]) // 128
                            seq += [(h, j) for j in range(j0, j0 + int(tiles_bh[b][h]))]
                        for k, (h, j) in enumerate(seq):
                            nc.tensor.matmul(
                                pg[:, bi * 128 : (bi + 1) * 128],
                                mts[h][:, j, :], sws[h][:, j, :],
                                start=(k == 0), stop=False,
                            )
                        # self-loop term: out[:, d] += dinv2[d] * HP[:, d]
                        # via block-diagonal matmul (lhsT = this block's rows)
                        nc.tensor.matmul(
                            pg[:, bi * 128 : (bi + 1) * 128],
                            HPROWS[:, b, :], DIAG2[:, b * 128 : (b + 1) * 128],
                            start=False, stop=True,
                        )

                    # epilogue straight from PSUM: bias + leaky on ACT
                    if l < 2:
                        if LEAKY_VIA_PRELU:
                            nc.scalar.activation(
                                HOUT[:, g0 : g0 + gw], pg[:, :gw],
                                mybir.ActivationFunctionType.Prelu,
                                bias=B[l][:, 0:1], scale=1.0, alpha=NEG_SLOPE,
                            )
                        else:
                            t2 = wp.tile([128, 512], f32, tag="ep2")
                            nc.scalar.activation(
                                t2[:, :gw], pg[:, :gw],
                                mybir.ActivationFunctionType.Identity,
                                bias=B[l][:, 0:1], scale=1.0,
                            )
                            t3 = wp.tile([128, 512], f32, tag="ep3")
                            nc.vector.tensor_scalar_mul(
                                t3[:, :gw], t2[:, :gw], NEG_SLOPE
                            )
                            nc.vector.tensor_tensor(
                                HOUT[:, g0 : g0 + gw], t2[:, :gw], t3[:, :gw],
                                op=mybir.AluOpType.max,
                            )
                        # fused next-layer producer for this chunk
                        ph = psd.tile([128, 512], f32, tag="pd")
                        nc.tensor.matmul(
                            ph[:, :gw], W[l + 1][:, :], HOUT[:, g0 : g0 + gw],
                            start=True, stop=True,
                        )
                        hpf = wp.tile([128, 512], f32, tag="hpf")
                        act_copy(hpf[:, :gw], ph[:, :gw])
                        producer_group(g, l + 1, hpf, g0)
                    else:
                        nc.scalar.activation(
                            HOUT[:, g0 : g0 + gw], pg[:, :gw],
                            mybir.ActivationFunctionType.Identity,
                            bias=B[l][:, 0:1], scale=1.0,
                        )
                        producer_group(g, l, HOUT[:, g0 : g0 + gw], g0, final=True)

                if l < 2:
                    all_gather(1, l + 1)

    nc.compile()
    return nc


_CACHE = {}


def kernel(
    x,
    edge_index,
    edge_attr,
    edge_type,
    edge_type_scale,
    W1,
    b1,
    W2,
    b2,
    W3,
    b3,
):
    x = np.asarray(x)
    N = x.shape[0]
    meta, per_core = _preprocess(
        np.asarray(x), np.asarray(edge_index), np.asarray(edge_attr),
        np.asarray(edge_type), np.asarray(edge_type_scale),
    )

    key = (N, meta["T"], tuple(meta["call_cnt"]))
    if key not in _CACHE:
        _CACHE[key] = _build(meta)
    nc = _CACHE[key]

    iota_f = np.tile(np.arange(128, dtype=np.float32)[None, :], (128, 1))
    ident = np.eye(128, dtype=np.float32)
    common = dict(
        W1=np.asarray(W1, np.float32),
        W2=np.asarray(W2, np.float32),
        W3=np.asarray(W3, np.float32),
        b1=np.asarray(b1, np.float32).reshape(D, 1),
        b2=np.asarray(b2, np.float32).reshape(D, 1),
        b3=np.asarray(b3, np.float32).reshape(D, 1),
        IOTAB=iota_f.astype(BF16),
        IDENT=ident,
        IDENTB=ident.astype(BF16),
    )
    in_maps = []
    for c in range(N_CORES):
        m = dict(common)
        m.update(per_core[c])
        in_maps.append(m)

    res = run_bass_kernel_spmd(
        nc, in_maps, core_ids=list(range(N_CORES)), **_RUN_KWARGS
    )
    _LAST_RESULT.clear()
    _LAST_RESULT["exec_time_ns"] = res.exec_time_ns
    _LAST_RESULT["profile_json"] = res.profile_json
    out = np.concatenate([res.results[c]["OUT"] for c in range(N_CORES)], axis=0)
    return out.astype(np.float32)


_RUN_KWARGS = {}  # test harness can set {"trace": True, "tmpdir": ...}
_LAST_RESULT = {}
